# revision 5
# baseline (speedup 1.0000x reference)
"""LowRankSparse2to4Linear Trainium2 kernel.

out = (x16 @ A16) -> fp16 -> (@ B16^T) + bias, where A16/B16 are the 2:4
soft-thresholded (along rank), scaled, fp16-cast low-rank factors.

Strategy (8 NeuronCores, data-parallel over tokens):
  - tokens (8192) sharded 1024/core; weights sharded for PREPROCESSING only
    (512 rows each of weight_A / weight_B per core), 2:4 soft-threshold done
    on-chip in fp16 on deinterleaved (rank-permuted) tiles, then two
    AllGathers distribute the fp16 preprocessed weights to every core.
    The rank permutation is consistent between A and B^T so it cancels in
    the second GEMM's contraction.
  - GEMM1 computes x_proj^T = A_sp^T @ x^T (rank-major) so its output feeds
    GEMM2 as the stationary operand without any transpose.
  - x tiles are fp16-cast (scalar engine) and transposed on the tensor
    engine; weight_B shard is transposed on the tensor engine before the
    AllGather.
"""

import os
import sys
import numpy as np

sys.path.insert(0, "/opt/trn_rl_repo")

N_CORES = 8
IN_F, OUT_F, RANK = 4096, 4096, 1024
T_FULL = 8192             # 4 * 2048 tokens
TPC = T_FULL // N_CORES   # 1024 tokens per core
WA_SH = IN_F // N_CORES   # 512 rows of weight_A per core
WB_SH = OUT_F // N_CORES  # 512 rows of weight_B per core

_BUILD_CACHE = {}


def _build(scale_a: float, scale_b: float, use_cc: bool = True):
    import concourse.bacc as bacc
    import concourse.tile as tile
    from concourse import mybir
    from concourse.masks import make_identity

    f32 = mybir.dt.float32
    f16 = mybir.dt.float16
    Alu = mybir.AluOpType
    AF = mybir.ActivationFunctionType

    nc = bacc.Bacc("TRN2", target_bir_lowering=False, debug=False,
                   num_devices=N_CORES)

    x_sh = nc.dram_tensor("x_sh", [TPC, IN_F], f32, kind="ExternalInput")
    wa_sh = nc.dram_tensor("wa_sh", [WA_SH, RANK], f32, kind="ExternalInput")
    wb_sh = nc.dram_tensor("wb_sh", [WB_SH, RANK], f32, kind="ExternalInput")
    bias_d = nc.dram_tensor("bias_d", [1, OUT_F], f32, kind="ExternalInput")
    out_d = nc.dram_tensor("out_d", [TPC, OUT_F], f32, kind="ExternalOutput")

    # collective bounce buffers (DRAM). AllGather stacks shards on axis 0.
    cc_a_in = nc.dram_tensor("cc_a_in", [WA_SH, RANK], f16)
    cc_b_in = nc.dram_tensor("cc_b_in", [RANK, WB_SH], f16)
    cc_a_out = nc.dram_tensor("cc_a_out", [IN_F, RANK], f16,
                              addr_space="Shared")
    cc_b_out = nc.dram_tensor("cc_b_out", [N_CORES * RANK, WB_SH], f16,
                              addr_space="Shared")

    K_IN = IN_F // 128    # 32 contraction chunks for GEMM1
    K_RK = RANK // 128    # 8 contraction chunks for GEMM2
    N_TOK = TPC // 128    # 8 token chunks per core

    with tile.TileContext(nc) as tc:
        with (
            tc.tile_pool(name="singles", bufs=1) as singles,
            tc.tile_pool(name="wst", bufs=2) as p_wst,
            tc.tile_pool(name="lane", bufs=10) as p_lane,
            tc.tile_pool(name="tmp", bufs=12) as p_tmp,
            tc.tile_pool(name="wspa", bufs=2) as p_wspa,
            tc.tile_pool(name="wspb", bufs=4) as p_wspb,
            tc.tile_pool(name="tbuf", bufs=2) as p_tb,
            tc.tile_pool(name="xf", bufs=3) as p_xf,
            tc.tile_pool(name="x16", bufs=4) as p_x16,
            tc.tile_pool(name="xt", bufs=34) as p_xt,
            tc.tile_pool(name="wastr", bufs=6) as p_wa,
            tc.tile_pool(name="xproj", bufs=16) as p_xp,
            tc.tile_pool(name="wbt", bufs=16) as p_wbt,
            tc.tile_pool(name="oev", bufs=4) as p_out,
            tc.tile_pool(name="psg1", bufs=4, space="PSUM") as p_psg1,
            tc.tile_pool(name="pssm", bufs=3, space="PSUM") as p_pssm,
        ):
            ident = singles.tile([128, 128], f16)
            make_identity(nc, ident[:])

            # ---------------- weight shard preprocessing ----------------
            def soft24(src_dram, scale, wsp_pool, nchunks):
                """2:4 soft-threshold a (nchunks*128, RANK) f32 shard into
                fp16 chunks in the deinterleaved rank layout:
                out[:, 256*i + q] = soft(w)[:, 4*q + i] * scale."""
                chunks = []
                for wc in range(nchunks):
                    st = p_wst.tile([128, RANK], f32, tag="wst")
                    nc.sync.dma_start(
                        st[:], src_dram[wc * 128:(wc + 1) * 128, :])
                    g3 = st[:].rearrange("p (q f) -> p q f", f=4)
                    lanes = []
                    for i in range(4):
                        ln = p_lane.tile([128, 256], f16, tag="lane")
                        # fp16 cast + scale fused; pre-scaling commutes with
                        # the soft threshold exactly.
                        nc.scalar.activation(ln[:], g3[:, :, i], AF.Copy,
                                             scale=float(scale))
                        lanes.append(ln)
                    mags = []
                    for i in range(4):
                        mi = p_lane.tile([128, 256], f16, tag="mag")
                        # |scale*w| fused: abs + scale + fp16 cast in one op
                        nc.scalar.activation(mi[:], g3[:, :, i], AF.Abs,
                                             scale=float(scale))
                        mags.append(mi)

                    def tt(op, a, b, tag="tmp"):
                        o = p_tmp.tile([128, 256], f16, tag=tag)
                        nc.vector.tensor_tensor(out=o[:], in0=a[:], in1=b[:],
                                                op=op)
                        return o

                    A = tt(Alu.min, mags[0], mags[1])
                    Bp = tt(Alu.min, mags[2], mags[3])
                    C = tt(Alu.max, mags[0], mags[1])
                    D = tt(Alu.max, mags[2], mags[3])
                    E = tt(Alu.max, A, Bp)
                    F = tt(Alu.min, C, D)
                    t = tt(Alu.min, E, F)           # 2nd-smallest magnitude
                    ntv = p_tmp.tile([128, 256], f16, tag="tmp")
                    nc.vector.tensor_scalar(ntv[:], t[:], -1.0, None, Alu.mult)

                    wsp = wsp_pool.tile([128, RANK], f16)
                    for i in range(4):
                        lo = tt(Alu.max, lanes[i], ntv)
                        cl = tt(Alu.min, lo, t)
                        # sp = g - clamp(g, -t, t)
                        nc.vector.tensor_tensor(
                            out=wsp[:, i * 256:(i + 1) * 256],
                            in0=lanes[i][:], in1=cl[:], op=Alu.subtract)
                    chunks.append(wsp)
                return chunks

            wa_chunks = soft24(wa_sh, scale_a, p_wspa, WA_SH // 128)
            for wc, wsp in enumerate(wa_chunks):
                nc.sync.dma_start(cc_a_in[wc * 128:(wc + 1) * 128, :], wsp[:])

            wb_chunks = soft24(wb_sh, scale_b, p_wspb, WB_SH // 128)
            # transpose the wb shard: (512 out, RANK) -> (RANK, 512 out)
            for rk in range(K_RK):
                pt = p_pssm.tile([128, 512], f16, tag="ps")
                for wc in range(4):
                    nc.tensor.transpose(
                        pt[:, wc * 128:(wc + 1) * 128],
                        wb_chunks[wc][:, rk * 128:(rk + 1) * 128],
                        ident[:])
                tb = p_tb.tile([128, 512], f16)
                nc.scalar.copy(tb[:], pt[:])
                nc.sync.dma_start(cc_b_in[rk * 128:(rk + 1) * 128, :], tb[:])

            # ---------------- AllGather preprocessed weights -------------
            if use_cc:
                nc.gpsimd.collective_compute(
                    "AllGather", Alu.bypass,
                    replica_groups=[list(range(N_CORES))],
                    ins=[cc_a_in[:]], outs=[cc_a_out[:]])
                nc.gpsimd.collective_compute(
                    "AllGather", Alu.bypass,
                    replica_groups=[list(range(N_CORES))],
                    ins=[cc_b_in[:]], outs=[cc_b_out[:]])
            else:
                # profiling-only variant: fake the gather with local copies
                for c in range(N_CORES):
                    nc.sync.dma_start(
                        cc_a_out[c * WA_SH:(c + 1) * WA_SH, :], cc_a_in[:])
                    nc.sync.dma_start(
                        cc_b_out[c * RANK:(c + 1) * RANK, :], cc_b_in[:])

            # ---------------- bias broadcast to 128 partitions -----------
            bias_row = singles.tile([1, OUT_F], f32)
            nc.sync.dma_start(bias_row[:], bias_d[:])
            ones_t = singles.tile([1, 128], f32)
            nc.gpsimd.memset(ones_t[:], 1.0)
            bias_bc = singles.tile([128, OUT_F], f32)
            for nb in range(OUT_F // 512):
                pb = p_pssm.tile([128, 512], f32, tag="ps")
                nc.tensor.matmul(pb[:], ones_t[:],
                                 bias_row[:, nb * 512:(nb + 1) * 512],
                                 start=True, stop=True)
                nc.vector.tensor_copy(
                    out=bias_bc[:, nb * 512:(nb + 1) * 512], in_=pb[:])

            # ---------------- x pipeline + GEMM1, per token-half ---------
            xproj = {}  # (th, rank_chunk) -> (128, 512) fp16 tile
            for th in range(2):
                # load + fp16-cast the 4 token chunks of this half
                x16s = []
                for tc4 in range(4):
                    tok0 = (th * 4 + tc4) * 128
                    xt16 = p_x16.tile([128, IN_F], f16, tag="x16")
                    for h in range(2):
                        xf = p_xf.tile([128, IN_F // 2], f32, tag="xf")
                        nc.sync.dma_start(
                            xf[:],
                            x_sh[tok0:tok0 + 128,
                                 h * (IN_F // 2):(h + 1) * (IN_F // 2)])
                        nc.scalar.copy(
                            xt16[:, h * (IN_F // 2):(h + 1) * (IN_F // 2)],
                            xf[:])
                    x16s.append(xt16)
                # transpose: xT[ic] = (128 in, 512 tok) fp16
                xT = []
                for ic in range(K_IN):
                    pt = p_pssm.tile([128, 512], f16, tag="ps")
                    for tc4 in range(4):
                        nc.tensor.transpose(
                            pt[:, tc4 * 128:(tc4 + 1) * 128],
                            x16s[tc4][:, ic * 128:(ic + 1) * 128],
                            ident[:])
                    xt = p_xt.tile([128, 512], f16, tag="xt")
                    nc.scalar.copy(xt[:], pt[:])
                    xT.append(xt)
                # GEMM1: x_proj^T[rank, tok] += wa^T @ x^T, rank in 2 sweeps
                for mh in range(2):
                    accs = [p_psg1.tile([128, 512], f32, tag="g1",
                                        name=f"g1acc_{th}_{mh}_{m}")
                            for m in range(4)]
                    for ic in range(K_IN):
                        wv = p_wa.tile([128, 512], f16, tag="wa")
                        nc.sync.dma_start(
                            wv[:],
                            cc_a_out[ic * 128:(ic + 1) * 128,
                                     mh * 512:(mh + 1) * 512])
                        for m in range(4):
                            nc.tensor.matmul(
                                accs[m][:],
                                wv[:, m * 128:(m + 1) * 128],
                                xT[ic][:],
                                start=(ic == 0), stop=(ic == K_IN - 1))
                    for m in range(4):
                        xp = p_xp.tile([128, 512], f16, tag="xp")
                        nc.scalar.copy(xp[:], accs[m][:])
                        xproj[(th, mh * 4 + m)] = xp

            # ---------------- GEMM2 + bias + store -----------------------
            for nb in range(OUT_F // 512):
                wbts = []
                for kc in range(K_RK):
                    wt = p_wbt.tile([128, 512], f16, tag="wbt")
                    nc.sync.dma_start(
                        wt[:],
                        cc_b_out[nb * RANK + kc * 128:
                                 nb * RANK + (kc + 1) * 128, :])
                    wbts.append(wt)
                for mt in range(N_TOK):
                    acc2 = p_pssm.tile([128, 512], f32, tag="ps")
                    th, ml = mt // 4, mt % 4
                    for kc in range(K_RK):
                        nc.tensor.matmul(
                            acc2[:],
                            xproj[(th, kc)][:, ml * 128:(ml + 1) * 128],
                            wbts[kc][:],
                            start=(kc == 0), stop=(kc == K_RK - 1))
                    ot = p_out.tile([128, 512], f32, tag="oev")
                    nc.vector.tensor_tensor(
                        out=ot[:], in0=acc2[:],
                        in1=bias_bc[:, nb * 512:(nb + 1) * 512],
                        op=Alu.add)
                    nc.sync.dma_start(
                        out_d[mt * 128:(mt + 1) * 128,
                              nb * 512:(nb + 1) * 512],
                        ot[:])

    nc.compile()
    return nc


def kernel(x, weight_A, weight_B, bias, scale_A, scale_B):
    from concourse.bass_utils import run_bass_kernel_spmd

    x = np.ascontiguousarray(np.asarray(x, dtype=np.float32))
    weight_A = np.ascontiguousarray(np.asarray(weight_A, dtype=np.float32))
    weight_B = np.ascontiguousarray(np.asarray(weight_B, dtype=np.float32))
    bias = np.ascontiguousarray(np.asarray(bias, dtype=np.float32))
    sa = float(np.asarray(scale_A))
    sb = float(np.asarray(scale_B))

    lead = x.shape[:-1]
    xf = x.reshape(-1, IN_F)
    assert xf.shape == (T_FULL, IN_F)

    use_cc = os.environ.get("BASS_KERNEL_NOCC", "0") != "1"
    key = (sa, sb, use_cc)
    if key not in _BUILD_CACHE:
        _BUILD_CACHE[key] = _build(sa, sb, use_cc)
    nc = _BUILD_CACHE[key]

    bias_row = bias.reshape(1, OUT_F)
    in_maps = []
    for c in range(N_CORES):
        in_maps.append({
            "x_sh": xf[c * TPC:(c + 1) * TPC],
            "wa_sh": weight_A[c * WA_SH:(c + 1) * WA_SH],
            "wb_sh": weight_B[c * WB_SH:(c + 1) * WB_SH],
            "bias_d": bias_row,
        })

    trace = os.environ.get("BASS_KERNEL_TRACE", "0") == "1"
    kwargs = {}
    if trace:
        _install_ntff_hook()
        kwargs["trace"] = True
        tmpdir = os.environ.get("BASS_KERNEL_TRACE_DIR")
        if tmpdir:
            os.makedirs(tmpdir, exist_ok=True)
            kwargs["tmpdir"] = tmpdir

    res = run_bass_kernel_spmd(nc, in_maps, core_ids=list(range(N_CORES)),
                               **kwargs)
    if trace:
        kernel.last_exec_time_ns = res.exec_time_ns

    out = np.empty((T_FULL, OUT_F), dtype=np.float32)
    for c in range(N_CORES):
        out[c * TPC:(c + 1) * TPC] = res.results[c]["out_d"]
    return out.reshape(*lead, OUT_F)


def _install_ntff_hook():
    """Provide antenv.axon_hooks (missing in this image) so trace=True works."""
    import types
    if "antenv.axon_hooks" in sys.modules:
        return
    try:
        from trn_agent_boot.trn_boot import _ntff_profile_via_ctypes
        hook = _ntff_profile_via_ctypes("/opt/axon/libaxon_pjrt.so")
    except Exception:
        hook = None
    mod = types.ModuleType("antenv.axon_hooks")
    mod.get_axon_ntff_profile_hook = lambda: hook
    mod.set_axon_ntff_profile_hook = lambda h: None
    import antenv  # noqa: F401
    sys.modules["antenv.axon_hooks"] = mod
